# revision 1
# baseline (speedup 1.0000x reference)
"""DeepSet segment-reduce kernel for 8 Trainium2 NeuronCores.

Math (equivalent to the reference, using linearity of segment_sum):
    r      = relu(x @ W1 + b1)                      # per-node, on device
    sums_r = segment_sum(r)                         # [B, HID]
    mean_r = sums_r / max(counts, 1)                # counts via host bincount
    hid    = mean_r @ W2 + b2 * (counts > 0)        # tiny tail, on device
    out    = relu(hid @ W3 + b3) @ W4 + b4          # tiny tail, on device

Device layout: hid on partitions, nodes on the free dim.  Each core gets a
contiguous shard of nodes split into two halves packed on partition halves
(features of half A in partitions 0..63, half B in 64..127), so DMA runs at
full 128-partition width and the two K=64 matmuls run concurrently in
different PE row groups with W1 resident in both halves of the array.

Segment sums: the host reorders each half so every segment's node run is
zero-padded to a multiple of 512 columns.  Every 512-column tile then
belongs to exactly one segment, and a fused relu+accumulate (ACT
``activation`` with ``accum_out``) per tile produces exact per-tile sums S
with no boundary fixups.  S is PE-transposed and multiplied by a per-core
0/1 routing matrix A (tile -> segment, host data) to form the per-core
partial segment sums.  The host adds the 8 partial results (and removes the
pad columns' relu(b1) contribution), then a second tiny NEFF applies the
mean and the rho MLP.
"""

import os
import sys

for _p in ("/opt/trn_rl_repo",):
    if os.path.isdir(_p) and _p not in sys.path:
        sys.path.append(_p)

import numpy as np

import concourse.bass as bass
import concourse.tile as tile
from concourse import bacc, mybir
from concourse.bass_utils import run_bass_kernel_spmd

F32 = mybir.dt.float32
F32R = mybir.dt.float32r

NCORES = 8
TILE = 512
DC = 5              # tiles per DMA chunk
NSEG = 1024
ODIM = 16


def _host_prep(x, x_batch, W1, b1, ncores=NCORES):
    N, _ = x.shape
    assert N % (2 * ncores) == 0
    ch = N // (2 * ncores)
    xb = np.asarray(x_batch)

    counts = np.bincount(xb, minlength=NSEG).astype(np.float64)

    plans = []          # (core, half) -> (src, tile_seg)
    cols_needed = 0
    for c in range(ncores):
        for h in range(2):
            lo = (2 * c + h) * ch
            ids = xb[lo:lo + ch]
            uniq, starts = np.unique(ids, return_index=True)
            ends = np.append(starts[1:], ch)
            src = np.full(ch + (len(uniq) + 1) * TILE, -1, dtype=np.int64)
            tile_seg = []
            col = 0
            for k in range(len(uniq)):
                L = int(ends[k] - starts[k])
                T = -(-L // TILE)
                src[col:col + L] = np.arange(lo + starts[k], lo + ends[k])
                tile_seg += [int(uniq[k])] * T
                col += T * TILE
            plans.append((src, tile_seg, col))
            cols_needed = max(cols_needed, col)

    ct = -(-cols_needed // TILE)
    cols = ct * TILE
    sw = -(-2 * ct // 128) * 128          # S width, multiple of 128
    nch = sw // 128

    padcount = np.zeros(NSEG, dtype=np.float64)
    in_maps = []
    w1d = np.vstack([W1, W1]).astype(np.float32)
    ident = np.eye(128, dtype=np.float32)
    for c in range(ncores):
        xt = np.zeros((128, cols), dtype=np.float32)
        amat = np.zeros((sw, NSEG), dtype=np.float32)
        for h in range(2):
            src, tile_seg, col = plans[2 * c + h]
            src = src[:col]
            if len(src) < cols:
                src = np.concatenate([src, np.full(cols - len(src), -1, np.int64)])
            else:
                src = src[:cols]
            mask = src >= 0
            gath = np.zeros((cols, 64), dtype=np.float32)
            gath[mask] = x[src[mask]]
            xt[64 * h:64 * h + 64, :] = gath.T
            for t, seg in enumerate(tile_seg):
                amat[h * ct + t, seg] = 1.0
            if tile_seg:
                seg_arr = np.array(tile_seg, dtype=np.int64)
                real = (src[:col] >= 0).reshape(-1, TILE).sum(axis=1)
                np.add.at(padcount, seg_arr, TILE - real[:len(seg_arr)])
        in_maps.append(dict(xt=xt, w1d=w1d, ident=ident, amat=amat,
                            b1=np.ascontiguousarray(b1, np.float32).reshape(128, 1)))

    meta = dict(ct=ct, cols=cols, sw=sw, nch=nch, ncores=ncores,
                counts=counts, padcount=padcount)
    return in_maps, meta


def _build_phase1(meta):
    ct, cols, sw, nch = meta["ct"], meta["cols"], meta["sw"], meta["nch"]
    ncores = meta["ncores"]

    nc = bacc.Bacc("TRN2", target_bir_lowering=False, debug=False,
                   num_devices=ncores)
    xt_d = nc.dram_tensor("xt", [128, cols], F32R, kind="ExternalInput").ap()
    w1_d = nc.dram_tensor("w1d", [128, 128], F32R, kind="ExternalInput").ap()
    b1_d = nc.dram_tensor("b1", [128, 1], F32, kind="ExternalInput").ap()
    id_d = nc.dram_tensor("ident", [128, 128], F32, kind="ExternalInput").ap()
    am_d = nc.dram_tensor("amat", [sw, NSEG], F32, kind="ExternalInput").ap()
    ps_d = nc.dram_tensor("psums", [128, NSEG], F32, kind="ExternalOutput").ap()

    with tile.TileContext(nc) as tc:
        with tc.tile_pool(name="const", bufs=1) as cpool, \
             tc.tile_pool(name="xin", bufs=3) as xpool, \
             tc.tile_pool(name="tr", bufs=4) as trpool, \
             tc.tile_pool(name="ps", bufs=5, space="PSUM") as pspool, \
             tc.tile_pool(name="psc", bufs=1, space="PSUM") as pscpool:

            w1t = cpool.tile([128, 128], F32R)
            nc.sync.dma_start(w1t[:], w1_d[:])
            b1t = cpool.tile([128, 1], F32)
            nc.sync.dma_start(b1t[:], b1_d[:])
            ident = cpool.tile([128, 128], F32)
            nc.sync.dma_start(ident[:], id_d[:])
            amat = cpool.tile([128, nch, NSEG], F32)
            nc.sync.dma_start(amat[:], am_d.rearrange("(k p) s -> p k s", p=128))
            S = cpool.tile([128, sw], F32)
            nc.vector.memset(S[:], 0.0)

            xtile = None
            for t in range(ct):
                if t % DC == 0:
                    w = min(DC, ct - t) * TILE
                    xtile = xpool.tile([128, DC * TILE], F32R, tag="xt")
                    nc.sync.dma_start(xtile[:, :w], xt_d[:, t * TILE:t * TILE + w])
                off = (t % DC) * TILE
                for half in range(2):
                    ps = pspool.tile([128, TILE], F32)
                    nc.tensor.matmul(
                        ps[:],
                        lhsT=w1t[64 * half:64 * half + 64, :],
                        rhs=xtile[64 * half:64 * half + 64, off:off + TILE],
                        start=True, stop=True,
                    )
                    trash = trpool.tile([128, TILE], F32, tag="trash")
                    s_col = half * ct + t
                    nc.scalar.activation(
                        out=trash[:], in_=ps[:],
                        func=mybir.ActivationFunctionType.Relu,
                        bias=b1t[:, 0:1],
                        accum_out=S[:, s_col:s_col + 1],
                    )

            # transpose S into [tiles, hid] chunks and combine with amat
            st_chunks = []
            for k in range(nch):
                pst = pscpool.tile([128, 128], F32, tag="pst")
                nc.tensor.transpose(pst[:], S[:, 128 * k:128 * k + 128], ident[:])
                st = trpool.tile([128, 128], F32, tag=f"st{k}")
                nc.vector.tensor_copy(st[:], pst[:])
                st_chunks.append(st)
            out_sb = trpool.tile([128, NSEG], F32, tag="out_sb")
            for j in range(NSEG // 512):
                pss = pscpool.tile([128, 512], F32, tag="pss")
                for k in range(nch):
                    nc.tensor.matmul(
                        pss[:], lhsT=st_chunks[k][:],
                        rhs=amat[:, k, 512 * j:512 * j + 512],
                        start=(k == 0), stop=(k == nch - 1),
                    )
                nc.vector.tensor_copy(out_sb[:, 512 * j:512 * j + 512], pss[:])
            nc.sync.dma_start(ps_d[:], out_sb[:])

    nc.compile()
    return nc


def _build_phase2():
    nc = bacc.Bacc("TRN2", target_bir_lowering=False, debug=False, num_devices=1)
    gs_d = nc.dram_tensor("gsums", [128, NSEG], F32, kind="ExternalInput").ap()
    invc_d = nc.dram_tensor("invc", [128, NSEG], F32, kind="ExternalInput").ap()
    b2nz_d = nc.dram_tensor("b2nz", [128, NSEG], F32, kind="ExternalInput").ap()
    w2_d = nc.dram_tensor("w2", [128, 128], F32, kind="ExternalInput").ap()
    w3_d = nc.dram_tensor("w3", [128, 128], F32, kind="ExternalInput").ap()
    w4_d = nc.dram_tensor("w4", [128, ODIM], F32, kind="ExternalInput").ap()
    b3_d = nc.dram_tensor("b3", [128, 1], F32, kind="ExternalInput").ap()
    b4_d = nc.dram_tensor("b4", [ODIM, 1], F32, kind="ExternalInput").ap()
    out_d = nc.dram_tensor("out_t", [ODIM, NSEG], F32, kind="ExternalOutput").ap()

    with tile.TileContext(nc) as tc:
        with tc.tile_pool(name="sb", bufs=1) as pool, \
             tc.tile_pool(name="ps", bufs=2, space="PSUM") as psp:
            gs = pool.tile([128, NSEG], F32)
            nc.sync.dma_start(gs[:], gs_d[:])
            invc = pool.tile([128, NSEG], F32)
            nc.sync.dma_start(invc[:], invc_d[:])
            b2nz = pool.tile([128, NSEG], F32)
            nc.sync.dma_start(b2nz[:], b2nz_d[:])
            w2 = pool.tile([128, 128], F32)
            nc.sync.dma_start(w2[:], w2_d[:])
            w3 = pool.tile([128, 128], F32)
            nc.sync.dma_start(w3[:], w3_d[:])
            w4 = pool.tile([128, ODIM], F32)
            nc.sync.dma_start(w4[:], w4_d[:])
            b3 = pool.tile([128, 1], F32)
            nc.sync.dma_start(b3[:], b3_d[:])
            b4 = pool.tile([ODIM, 1], F32)
            nc.sync.dma_start(b4[:], b4_d[:])

            mean = pool.tile([128, NSEG], F32)
            nc.vector.tensor_tensor(out=mean[:], in0=gs[:], in1=invc[:],
                                    op=mybir.AluOpType.mult)
            hid = pool.tile([128, NSEG], F32)
            t3 = pool.tile([128, NSEG], F32)
            ot = pool.tile([ODIM, NSEG], F32)
            for j in range(NSEG // 512):
                sl = slice(512 * j, 512 * j + 512)
                p2 = psp.tile([128, 512], F32, tag="p")
                nc.tensor.matmul(p2[:], lhsT=w2[:], rhs=mean[:, sl],
                                 start=True, stop=True)
                nc.vector.tensor_tensor(out=hid[:, sl], in0=p2[:],
                                        in1=b2nz[:, sl], op=mybir.AluOpType.add)
            for j in range(NSEG // 512):
                sl = slice(512 * j, 512 * j + 512)
                p3 = psp.tile([128, 512], F32, tag="p")
                nc.tensor.matmul(p3[:], lhsT=w3[:], rhs=hid[:, sl],
                                 start=True, stop=True)
                nc.scalar.activation(out=t3[:, sl], in_=p3[:],
                                     func=mybir.ActivationFunctionType.Relu,
                                     bias=b3[:, 0:1])
            for j in range(NSEG // 512):
                sl = slice(512 * j, 512 * j + 512)
                p4f = psp.tile([128, 512], F32, tag="p")
                p4 = p4f[:ODIM, :]
                nc.tensor.matmul(p4, lhsT=w4[:], rhs=t3[:, sl],
                                 start=True, stop=True)
                nc.scalar.activation(out=ot[:, sl], in_=p4,
                                     func=mybir.ActivationFunctionType.Identity,
                                     bias=b4[:, 0:1])
            nc.sync.dma_start(out_d[:], ot[:])
    nc.compile()
    return nc


def run(inputs, ncores=NCORES, trace=False):
    x = np.asarray(inputs["x"], dtype=np.float32)
    xb = np.asarray(inputs["x_batch"])
    W1 = np.asarray(inputs["W1"], dtype=np.float32)
    b1 = np.asarray(inputs["b1"], dtype=np.float32)
    in_maps, meta = _host_prep(x, xb, W1, b1, ncores=ncores)

    nc1 = _build_phase1(meta)
    res1 = run_bass_kernel_spmd(nc1, in_maps, core_ids=list(range(ncores)),
                                trace=trace)
    gsums = np.zeros((128, NSEG), dtype=np.float64)
    for c in range(ncores):
        gsums += res1.results[c]["psums"].astype(np.float64)
    # remove the relu(b1) contribution of zero-pad columns
    gsums -= np.maximum(b1, 0.0)[:, None].astype(np.float64) * meta["padcount"][None, :]
    gsums = gsums.astype(np.float32)

    counts = meta["counts"]
    invc = (1.0 / np.maximum(counts, 1.0)).astype(np.float32)
    nz = (counts > 0).astype(np.float32)
    b2 = np.asarray(inputs["b2"], dtype=np.float32)
    p2_ins = [dict(
        gsums=gsums,
        invc=np.ascontiguousarray(np.broadcast_to(invc, (128, NSEG))),
        b2nz=np.ascontiguousarray(b2.reshape(128, 1) * nz[None, :]),
        w2=np.ascontiguousarray(inputs["W2"], dtype=np.float32),
        w3=np.ascontiguousarray(inputs["W3"], dtype=np.float32),
        w4=np.ascontiguousarray(inputs["W4"], dtype=np.float32),
        b3=np.ascontiguousarray(inputs["b3"], dtype=np.float32).reshape(128, 1),
        b4=np.ascontiguousarray(inputs["b4"], dtype=np.float32).reshape(ODIM, 1),
    )]
    nc2 = _build_phase2()
    res2 = run_bass_kernel_spmd(nc2, p2_ins, core_ids=[0], trace=trace)
    out = np.ascontiguousarray(res2.results[0]["out_t"].T).astype(np.float32)
    return out, res1, res2


def kernel(**inputs):
    inputs = {k: np.asarray(v) for k, v in inputs.items()}
    out, _, _ = run(inputs)
    return out


if __name__ == "__main__":
    rng = np.random.default_rng(0)
    N, D, HN, B = 8 * 2 * 2 * TILE, 64, 128, 64
    x = rng.standard_normal((N, D), dtype=np.float32)
    xb = np.sort(rng.integers(0, B, N).astype(np.int32))
    W1 = (rng.standard_normal((D, HN)) / 8).astype(np.float32)
    W2 = (rng.standard_normal((HN, HN)) / 11.3).astype(np.float32)
    W3 = (rng.standard_normal((HN, HN)) / 11.3).astype(np.float32)
    W4 = (rng.standard_normal((HN, ODIM)) / 11.3).astype(np.float32)
    b1 = rng.standard_normal(HN).astype(np.float32) * 0.1
    b2 = rng.standard_normal(HN).astype(np.float32) * 0.1
    b3 = rng.standard_normal(HN).astype(np.float32) * 0.1
    b4 = rng.standard_normal(ODIM).astype(np.float32) * 0.1
    ins = dict(x=x, x_batch=xb, W1=W1, b1=b1, W2=W2, b2=b2, W3=W3, b3=b3,
               W4=W4, b4=b4)
    out = kernel(**ins)

    h = np.maximum(x @ W1 + b1, 0) @ W2 + b2
    sums = np.zeros((1024, HN), dtype=np.float64)
    np.add.at(sums, xb, h.astype(np.float64))
    cnt = np.bincount(xb, minlength=1024).astype(np.float64)
    mean = sums / np.maximum(cnt, 1)[:, None]
    ref = (np.maximum(mean @ W3 + b3, 0) @ W4 + b4).astype(np.float32)
    num = np.linalg.norm(out - ref)
    den = np.linalg.norm(ref)
    print("Relative error:", num / den)



# revision 3
# speedup vs baseline: 1.8284x; 1.8284x over previous
"""DeepSet segment-reduce kernel for 8 Trainium2 NeuronCores.

Math (equivalent to the reference, using linearity of segment_sum):
    r      = relu(x @ W1 + b1)                      # per-node, on device
    sums_r = segment_sum(r)                         # [B, HID]
    mean_r = sums_r / max(counts, 1)                # counts via host bincount
    hid    = mean_r @ W2 + b2                       # tiny tail, on device
    out    = relu(hid @ W3 + b3) @ W4 + b4          # tiny tail, on device

Device layout: hid on partitions, nodes on the free dim.  Each core gets a
contiguous shard of nodes split into two halves packed on partition halves
(features of half A in partitions 0..63, half B in 64..127), so DMA runs at
full 128-partition width.  Inputs are cast to fp16 on the host: the PE
streams fp16 at 1 col/cycle (4x the fp32 rate) and DMA traffic halves.

Segment sums: the host reorders each half so every segment's node run is
zero-padded to a multiple of 1024 columns.  Every 1024-column block then
belongs to exactly one segment.  Per block, two 512-col matmuls (PSUM bank
limit) produce h = W1^T x in PSUM, then ONE fused instruction computes the
block's relu sum into a column of S:
  - ScalarE blocks: activation(Relu, bias=b1, accum_out=S[:,col])
  - VectorE blocks: tensor_scalar(max(h, -b1), reduce add, accum_out)
    using relu(h + b1) = max(h, -b1) + b1; the  +b1*real_count  and the
    zero-pad columns' contributions are corrected exactly on the host.
The two engines work alternating blocks in disjoint PSUM bank regions
(ScalarE banks 0-3, VectorE banks 4-7), in parallel with the PE and DMA.
S [128, 2*nbh] is DMA'd out; segment routing, the cross-core reduction,
bias corrections and the mean all happen on the host (exact, cheap).
A second tiny single-core NEFF applies the rho MLP.
"""

import os
import sys

for _p in ("/opt/trn_rl_repo",):
    if os.path.isdir(_p) and _p not in sys.path:
        sys.path.append(_p)

import numpy as np

import concourse.bass as bass
import concourse.tile as tile
from concourse import bacc, mybir
from concourse.bass_utils import run_bass_kernel_spmd

F32 = mybir.dt.float32
F16 = mybir.dt.float16

NCORES = 8
GW = 1024           # columns per elementwise block (2 PSUM banks)
MW = 512            # columns per matmul (1 PSUM bank)
CW = 8192           # columns per DMA chunk
NSEG = 1024
ODIM = 16
HID = 128
NDIM = 64


def _engine_of(h, gb):
    """Block (half h, global block index gb) -> 0 (ScalarE) or 1 (VectorE).

    Must be identical between host prep and kernel build, and depend only on
    position (the SPMD NEFF is shared by all cores)."""
    return (gb + h) % 2


def _host_prep(x, x_batch, W1, b1, ncores=NCORES):
    N, _ = x.shape
    assert N % (2 * ncores) == 0
    ch = N // (2 * ncores)
    xb = np.asarray(x_batch)

    counts = np.bincount(xb, minlength=NSEG).astype(np.float64)

    # Per half: run-length decomposition, block plan
    plans = []          # (core,half) -> (starts,ends,uniq)
    cols_needed = 0
    for hh in range(2 * ncores):
        lo = hh * ch
        ids = xb[lo:lo + ch]
        uniq, starts = np.unique(ids, return_index=True)
        ends = np.append(starts[1:], ch)
        lens = ends - starts
        nblocks = -(-lens // GW)
        cols_needed = max(cols_needed, int(nblocks.sum()) * GW)
        plans.append((lo, uniq, starts, ends))

    cols = -(-cols_needed // CW) * CW
    nbh = cols // GW
    sw = 2 * nbh

    w1d = np.vstack([W1, W1]).astype(np.float16)
    b1f = np.ascontiguousarray(b1, np.float32).reshape(HID, 1)
    nb1f = np.ascontiguousarray(-b1, np.float32).reshape(HID, 1)

    in_maps = []
    block_maps = []     # per core: list of (s_col, seg, real, pad, eng)
    for c in range(ncores):
        xt = np.zeros((128, cols), dtype=np.float16)
        bmap = []
        for h in range(2):
            lo, uniq, starts, ends = plans[2 * c + h]
            src = np.full(cols, -1, dtype=np.int64)
            col = 0
            for k in range(len(uniq)):
                L = int(ends[k] - starts[k])
                nb = -(-L // GW)
                src[col:col + L] = np.arange(lo + starts[k], lo + ends[k])
                for j in range(nb):
                    gb = col // GW + j
                    real = min(GW, L - j * GW)
                    bmap.append((h * nbh + gb, int(uniq[k]), real,
                                 GW - real, _engine_of(h, gb)))
                col += nb * GW
            mask = src >= 0
            gath = np.zeros((cols, NDIM), dtype=np.float16)
            gath[mask] = x[src[mask]].astype(np.float16)
            xt[64 * h:64 * h + 64, :] = gath.T
        in_maps.append(dict(xt=xt, w1d=w1d, b1=b1f, nb1=nb1f))
        block_maps.append(bmap)

    meta = dict(cols=cols, nbh=nbh, sw=sw, ncores=ncores,
                counts=counts, block_maps=block_maps)
    return in_maps, meta


def _build_phase1(meta):
    cols, nbh, sw = meta["cols"], meta["nbh"], meta["sw"]
    ncores = meta["ncores"]
    nchunks = cols // CW
    bpc = CW // GW      # blocks per chunk per half

    nc = bacc.Bacc("TRN2", target_bir_lowering=False, debug=False,
                   num_devices=ncores)
    xt_d = nc.dram_tensor("xt", [128, cols], F16, kind="ExternalInput").ap()
    w1_d = nc.dram_tensor("w1d", [128, 128], F16, kind="ExternalInput").ap()
    b1_d = nc.dram_tensor("b1", [128, 1], F32, kind="ExternalInput").ap()
    nb1_d = nc.dram_tensor("nb1", [128, 1], F32, kind="ExternalInput").ap()
    s_d = nc.dram_tensor("S", [128, sw], F32, kind="ExternalOutput").ap()

    with tile.TileContext(nc) as tc:
        with tc.tile_pool(name="const", bufs=1) as cpool, \
             tc.tile_pool(name="xin", bufs=3) as xpool, \
             tc.tile_pool(name="sp", bufs=1) as spool, \
             tc.tile_pool(name="ps", bufs=1, space="PSUM") as pspool:

            w1t = cpool.tile([128, 128], F16)
            nc.sync.dma_start(w1t[:], w1_d[:])
            b1t = cpool.tile([128, 1], F32)
            nc.sync.dma_start(b1t[:], b1_d[:])
            nb1t = cpool.tile([128, 1], F32)
            nc.sync.dma_start(nb1t[:], nb1_d[:])
            S = spool.tile([128, sw], F32)
            ps = pspool.tile([128, 4096], F32)

            slot_ctr = [0, 0]   # per-engine ping-pong
            for ci in range(nchunks):
                xch = xpool.tile([128, CW], F16, tag="xch")
                nc.sync.dma_start(xch[:], xt_d[:, ci * CW:(ci + 1) * CW])
                for h in range(2):
                    for blk in range(bpc):
                        gb = ci * bpc + blk
                        s_col = h * nbh + gb
                        eng = _engine_of(h, gb)
                        slot = slot_ctr[eng] % 2
                        slot_ctr[eng] += 1
                        base = eng * 2048 + slot * GW
                        for s2 in range(GW // MW):
                            nc.tensor.matmul(
                                ps[:, base + s2 * MW: base + (s2 + 1) * MW],
                                lhsT=w1t[64 * h:64 * h + 64, :],
                                rhs=xch[64 * h:64 * h + 64,
                                        blk * GW + s2 * MW: blk * GW + (s2 + 1) * MW],
                                start=True, stop=True,
                            )
                        region = ps[:, base:base + GW]
                        if eng == 0:
                            nc.scalar.activation(
                                out=region, in_=region,
                                func=mybir.ActivationFunctionType.Relu,
                                bias=b1t[:, 0:1],
                                accum_out=S[:, s_col:s_col + 1],
                            )
                        else:
                            nc.vector.tensor_scalar(
                                region, region, nb1t[:, 0:1], None,
                                mybir.AluOpType.max, mybir.AluOpType.add,
                                accum_out=S[:, s_col:s_col + 1],
                            )
            nc.sync.dma_start(s_d[:], S[:])

    nc.compile()
    return nc


def _build_phase2():
    nc = bacc.Bacc("TRN2", target_bir_lowering=False, debug=False, num_devices=1)
    gm_d = nc.dram_tensor("gmean", [128, NSEG], F16, kind="ExternalInput").ap()
    w2_d = nc.dram_tensor("w2", [128, 128], F16, kind="ExternalInput").ap()
    w3_d = nc.dram_tensor("w3", [128, 128], F16, kind="ExternalInput").ap()
    w4_d = nc.dram_tensor("w4", [128, ODIM], F16, kind="ExternalInput").ap()
    b2_d = nc.dram_tensor("b2", [128, 1], F32, kind="ExternalInput").ap()
    b3_d = nc.dram_tensor("b3", [128, 1], F32, kind="ExternalInput").ap()
    b4_d = nc.dram_tensor("b4", [ODIM, 1], F32, kind="ExternalInput").ap()
    out_d = nc.dram_tensor("out_t", [ODIM, NSEG], F32, kind="ExternalOutput").ap()

    with tile.TileContext(nc) as tc:
        with tc.tile_pool(name="sb", bufs=1) as pool, \
             tc.tile_pool(name="ps", bufs=6, space="PSUM") as psp:
            gm = pool.tile([128, NSEG], F16)
            nc.sync.dma_start(gm[:], gm_d[:])
            w2 = pool.tile([128, 128], F16)
            nc.sync.dma_start(w2[:], w2_d[:])
            w3 = pool.tile([128, 128], F16)
            nc.sync.dma_start(w3[:], w3_d[:])
            w4 = pool.tile([128, ODIM], F16)
            nc.sync.dma_start(w4[:], w4_d[:])
            b2 = pool.tile([128, 1], F32)
            nc.sync.dma_start(b2[:], b2_d[:])
            b3 = pool.tile([128, 1], F32)
            nc.sync.dma_start(b3[:], b3_d[:])
            b4 = pool.tile([ODIM, 1], F32)
            nc.sync.dma_start(b4[:], b4_d[:])

            hid = pool.tile([128, NSEG], F16)
            t3 = pool.tile([128, NSEG], F16)
            ot = pool.tile([ODIM, NSEG], F32)
            for j in range(NSEG // 512):
                sl = slice(512 * j, 512 * j + 512)
                p2 = psp.tile([128, 512], F32, tag="p")
                nc.tensor.matmul(p2[:], lhsT=w2[:], rhs=gm[:, sl],
                                 start=True, stop=True)
                nc.scalar.activation(out=hid[:, sl], in_=p2[:],
                                     func=mybir.ActivationFunctionType.Identity,
                                     bias=b2[:, 0:1])
            for j in range(NSEG // 512):
                sl = slice(512 * j, 512 * j + 512)
                p3 = psp.tile([128, 512], F32, tag="p")
                nc.tensor.matmul(p3[:], lhsT=w3[:], rhs=hid[:, sl],
                                 start=True, stop=True)
                nc.scalar.activation(out=t3[:, sl], in_=p3[:],
                                     func=mybir.ActivationFunctionType.Relu,
                                     bias=b3[:, 0:1])
            for j in range(NSEG // 512):
                sl = slice(512 * j, 512 * j + 512)
                p4f = psp.tile([128, 512], F32, tag="p")
                p4 = p4f[:ODIM, :]
                nc.tensor.matmul(p4, lhsT=w4[:], rhs=t3[:, sl],
                                 start=True, stop=True)
                nc.scalar.activation(out=ot[:, sl], in_=p4,
                                     func=mybir.ActivationFunctionType.Identity,
                                     bias=b4[:, 0:1])
            nc.sync.dma_start(out_d[:], ot[:])
    nc.compile()
    return nc


def run(inputs, ncores=NCORES, trace=False):
    x = np.asarray(inputs["x"], dtype=np.float32)
    xb = np.asarray(inputs["x_batch"])
    W1 = np.asarray(inputs["W1"], dtype=np.float32)
    b1 = np.asarray(inputs["b1"], dtype=np.float32)
    in_maps, meta = _host_prep(x, xb, W1, b1, ncores=ncores)

    nc1 = _build_phase1(meta)
    res1 = run_bass_kernel_spmd(nc1, in_maps, core_ids=list(range(ncores)),
                                trace=trace)

    # Host: route block sums to segments, cross-core reduce, bias/pad fixups
    gsumsT = np.zeros((NSEG, 128), dtype=np.float64)
    padA = np.zeros(NSEG, dtype=np.float64)
    padD = np.zeros(NSEG, dtype=np.float64)
    realD = np.zeros(NSEG, dtype=np.float64)
    for c in range(ncores):
        Sc = res1.results[c]["S"].astype(np.float64)
        bmap = meta["block_maps"][c]
        s_cols = np.array([b[0] for b in bmap], dtype=np.int64)
        segs = np.array([b[1] for b in bmap], dtype=np.int64)
        np.add.at(gsumsT, segs, Sc[:, s_cols].T)
        for s_col, seg, real, pad, eng in bmap:
            if eng == 0:
                padA[seg] += pad
            else:
                padD[seg] += pad
                realD[seg] += real
    b1d = b1.astype(np.float64)
    gsumsT += np.outer(realD, b1d)                       # DVE real cols: +b1
    gsumsT -= np.outer(padA, np.maximum(b1d, 0.0))       # ACT pads: relu(b1)
    gsumsT -= np.outer(padD, np.maximum(-b1d, 0.0))      # DVE pads: relu(-b1)

    counts = meta["counts"]
    invc = 1.0 / np.maximum(counts, 1.0)
    gmean = (gsumsT * invc[:, None]).T.astype(np.float16)

    p2_ins = [dict(
        gmean=np.ascontiguousarray(gmean),
        w2=np.ascontiguousarray(inputs["W2"], dtype=np.float16),
        w3=np.ascontiguousarray(inputs["W3"], dtype=np.float16),
        w4=np.ascontiguousarray(inputs["W4"], dtype=np.float16),
        b2=np.ascontiguousarray(inputs["b2"], dtype=np.float32).reshape(128, 1),
        b3=np.ascontiguousarray(inputs["b3"], dtype=np.float32).reshape(128, 1),
        b4=np.ascontiguousarray(inputs["b4"], dtype=np.float32).reshape(ODIM, 1),
    )]
    nc2 = _build_phase2()
    res2 = run_bass_kernel_spmd(nc2, p2_ins, core_ids=[0], trace=trace)
    out = np.ascontiguousarray(res2.results[0]["out_t"].T).astype(np.float32)
    # Empty segments: reference gives hid = 0 (b2 enters per-node, so an
    # empty sum has no b2), but the device kernel adds b2 unconditionally.
    empty = counts == 0
    if empty.any():
        b3 = np.asarray(inputs["b3"], dtype=np.float64)
        W4 = np.asarray(inputs["W4"], dtype=np.float64)
        b4 = np.asarray(inputs["b4"], dtype=np.float64)
        out[empty] = (np.maximum(b3, 0.0) @ W4 + b4).astype(np.float32)
    return out, res1, res2


def kernel(**inputs):
    inputs = {k: np.asarray(v) for k, v in inputs.items()}
    out, _, _ = run(inputs)
    return out


if __name__ == "__main__":
    rng = np.random.default_rng(0)
    N, D, HN, B = 8 * 2 * 4 * GW, 64, 128, 64
    x = rng.standard_normal((N, D), dtype=np.float32)
    xb = np.sort(rng.integers(0, B, N).astype(np.int32))
    W1 = (rng.standard_normal((D, HN)) / 8).astype(np.float32)
    W2 = (rng.standard_normal((HN, HN)) / 11.3).astype(np.float32)
    W3 = (rng.standard_normal((HN, HN)) / 11.3).astype(np.float32)
    W4 = (rng.standard_normal((HN, ODIM)) / 11.3).astype(np.float32)
    b1 = rng.standard_normal(HN).astype(np.float32) * 0.1
    b2 = rng.standard_normal(HN).astype(np.float32) * 0.1
    b3 = rng.standard_normal(HN).astype(np.float32) * 0.1
    b4 = rng.standard_normal(ODIM).astype(np.float32) * 0.1
    ins = dict(x=x, x_batch=xb, W1=W1, b1=b1, W2=W2, b2=b2, W3=W3, b3=b3,
               W4=W4, b4=b4)
    out = kernel(**ins)

    h = np.maximum(x @ W1 + b1, 0) @ W2 + b2
    sums = np.zeros((1024, HN), dtype=np.float64)
    np.add.at(sums, xb, h.astype(np.float64))
    cnt = np.bincount(xb, minlength=1024).astype(np.float64)
    mean = sums / np.maximum(cnt, 1)[:, None]
    ref = (np.maximum(mean @ W3 + b3, 0) @ W4 + b4).astype(np.float32)
    num = np.linalg.norm(out - ref)
    den = np.linalg.norm(ref)
    print("Relative error:", num / den)


# revision 6
# speedup vs baseline: 1.8991x; 1.0387x over previous
"""DeepSet segment-reduce kernel for 8 Trainium2 NeuronCores.

Math (equivalent to the reference, using linearity of segment_sum):
    r      = relu(x @ W1 + b1)                      # per-node, on device
    sums_r = segment_sum(r)                         # [B, HID]
    mean_r = sums_r / max(counts, 1)                # counts via host bincount
    hid    = mean_r @ W2 + b2                       # tiny tail, on device
    out    = relu(hid @ W3 + b3) @ W4 + b4          # tiny tail, on device

Device layout: hid on partitions, nodes on the free dim.  Each core gets a
contiguous shard of nodes split into two halves packed on partition halves
(features of half A in partitions 0..63, half B in 64..127), so DMA runs at
full 128-partition width.  Inputs are cast to fp16 on the host: the PE
streams fp16 at 1 col/cycle (4x the fp32 rate) and DMA traffic halves.

Segment sums: the host reorders each half so every segment's node run is
zero-padded to a multiple of 1024 columns.  Every 1024-column block then
belongs to exactly one segment.  Per block, two 512-col matmuls (PSUM bank
limit) produce h = W1^T x in PSUM, then ONE fused instruction computes the
block's relu sum into a column of S:
  - ScalarE blocks: activation(Relu, bias=b1, accum_out=S[:,col])
  - VectorE blocks: tensor_scalar(max(h, -b1), reduce add, accum_out)
    using relu(h + b1) = max(h, -b1) + b1; the  +b1*real_count  and the
    zero-pad columns' contributions are corrected exactly on the host.
The two engines work alternating blocks in disjoint PSUM bank regions
(ScalarE banks 0-3, VectorE banks 4-7), in parallel with the PE and DMA.
S [128, 2*nbh] is DMA'd out; segment routing, the cross-core reduction,
bias corrections and the mean all happen on the host (exact, cheap).
A second tiny single-core NEFF applies the rho MLP.
"""

import os
import sys

for _p in ("/opt/trn_rl_repo",):
    if os.path.isdir(_p) and _p not in sys.path:
        sys.path.append(_p)

import numpy as np

import concourse.bass as bass
import concourse.tile as tile
from concourse import bacc, mybir
from concourse.bass_utils import run_bass_kernel_spmd

F32 = mybir.dt.float32
F16 = mybir.dt.float16

NCORES = 8
GW = 1024           # columns per elementwise block (2 PSUM banks)
MW = 512            # columns per matmul (1 PSUM bank)
CW = 8192           # columns per DMA chunk
NSEG = 1024
ODIM = 16
HID = 128
NDIM = 64


def _engine_of(h, gb):
    """Block (half h, global block index gb) -> 0 (ScalarE) or 1 (VectorE).

    Must be identical between host prep and kernel build, and depend only on
    position (the SPMD NEFF is shared by all cores)."""
    return (gb + h) % 2


def _host_prep(x, x_batch, W1, b1, ncores=NCORES):
    N, _ = x.shape
    xb = np.asarray(x_batch)

    counts = np.bincount(xb, minlength=NSEG).astype(np.float64)

    # Run-length decomposition of the (sorted) x_batch, then balanced
    # assignment of whole segments to the 16 (core, half) bins so every bin
    # has nearly the same padded block count.
    uniq, starts = np.unique(xb, return_index=True)
    ends = np.append(starts[1:], len(xb))
    lens = ends - starts
    nblk = -(-lens // GW)
    order = np.argsort(-nblk, kind="stable")
    nbins = 2 * ncores
    bin_blocks = np.zeros(nbins, dtype=np.int64)
    bin_segs = [[] for _ in range(nbins)]
    for k in order:
        b = int(np.argmin(bin_blocks))
        bin_segs[b].append(int(k))
        bin_blocks[b] += nblk[k]

    nbh = int(bin_blocks.max())
    cols = nbh * GW
    sw = 2 * nbh

    w1d = np.vstack([W1, W1]).astype(np.float16)
    b1f = np.ascontiguousarray(b1, np.float32).reshape(HID, 1)
    nb1f = np.ascontiguousarray(-b1, np.float32).reshape(HID, 1)

    in_maps = []
    block_maps = []     # per core: list of (s_col, seg, real, pad, eng)
    for c in range(ncores):
        xt = np.zeros((128, cols), dtype=np.float16)
        bmap = []
        for h in range(2):
            src = np.full(cols, -1, dtype=np.int64)
            col = 0
            for k in bin_segs[2 * c + h]:
                L = int(lens[k])
                nb = -(-L // GW)
                src[col:col + L] = np.arange(starts[k], ends[k])
                for j in range(nb):
                    gb = col // GW + j
                    real = min(GW, L - j * GW)
                    bmap.append((h * nbh + gb, int(uniq[k]), real,
                                 GW - real, _engine_of(h, gb)))
                col += nb * GW
            mask = src >= 0
            gath = np.zeros((cols, NDIM), dtype=np.float16)
            gath[mask] = x[src[mask]].astype(np.float16)
            xt[64 * h:64 * h + 64, :] = gath.T
        in_maps.append(dict(xt=xt, w1d=w1d, b1=b1f, nb1=nb1f))
        block_maps.append(bmap)

    meta = dict(cols=cols, nbh=nbh, sw=sw, ncores=ncores,
                counts=counts, block_maps=block_maps)
    return in_maps, meta


def _build_phase1(meta):
    cols, nbh, sw = meta["cols"], meta["nbh"], meta["sw"]
    ncores = meta["ncores"]
    cw_list = [CW] * (cols // CW)
    if cols % CW:
        cw_list.append(cols % CW)

    nc = bacc.Bacc("TRN2", target_bir_lowering=False, debug=False,
                   num_devices=ncores)
    xt_d = nc.dram_tensor("xt", [128, cols], F16, kind="ExternalInput").ap()
    w1_d = nc.dram_tensor("w1d", [128, 128], F16, kind="ExternalInput").ap()
    b1_d = nc.dram_tensor("b1", [128, 1], F32, kind="ExternalInput").ap()
    nb1_d = nc.dram_tensor("nb1", [128, 1], F32, kind="ExternalInput").ap()
    s_d = nc.dram_tensor("S", [128, sw], F32, kind="ExternalOutput").ap()

    with tile.TileContext(nc) as tc:
        with tc.tile_pool(name="const", bufs=1) as cpool, \
             tc.tile_pool(name="xin", bufs=3) as xpool, \
             tc.tile_pool(name="sp", bufs=1) as spool, \
             tc.tile_pool(name="ps", bufs=1, space="PSUM") as pspool:

            w1t = cpool.tile([128, 128], F16)
            nc.sync.dma_start(w1t[:], w1_d[:])
            b1t = cpool.tile([128, 1], F32)
            nc.sync.dma_start(b1t[:], b1_d[:])
            nb1t = cpool.tile([128, 1], F32)
            nc.sync.dma_start(nb1t[:], nb1_d[:])
            S = spool.tile([128, sw], F32)
            ps = pspool.tile([128, 4096], F32)

            # The PE array holds both halves' 64x128 weight tiles resident
            # for the whole kernel (disjoint row strips); every matmul
            # suppresses its implicit LDWEIGHTS, which otherwise serializes
            # with the ifmap stream on the xbus (~184ns per matmul).
            nc.tensor.ldweights(w1t[0:64, :], tile_position=(0, 0))
            nc.tensor.ldweights(w1t[64:128, :], tile_position=(64, 0))

            slot_ctr = [0, 0]   # per-engine ping-pong
            coff = 0
            for cw in cw_list:
                bpc = cw // GW
                xch = xpool.tile([128, CW], F16, tag="xch")
                nc.sync.dma_start(xch[:, :cw], xt_d[:, coff:coff + cw])
                for h in range(2):
                    for blk in range(bpc):
                        gb = coff // GW + blk
                        s_col = h * nbh + gb
                        eng = _engine_of(h, gb)
                        slot = slot_ctr[eng] % 2
                        slot_ctr[eng] += 1
                        base = eng * 2048 + slot * GW
                        for s2 in range(GW // MW):
                            mm = nc.tensor.matmul(
                                ps[:, base + s2 * MW: base + (s2 + 1) * MW],
                                lhsT=w1t[64 * h:64 * h + 64, :],
                                rhs=xch[64 * h:64 * h + 64,
                                        blk * GW + s2 * MW: blk * GW + (s2 + 1) * MW],
                                start=True, stop=True,
                                tile_position=(64 * h, 0),
                            )
                            mm.ins.ldweights = False
                        region = ps[:, base:base + GW]
                        if eng == 0:
                            nc.scalar.activation(
                                out=region, in_=region,
                                func=mybir.ActivationFunctionType.Relu,
                                bias=b1t[:, 0:1],
                                accum_out=S[:, s_col:s_col + 1],
                            )
                        else:
                            nc.vector.tensor_scalar(
                                region, region, nb1t[:, 0:1], None,
                                mybir.AluOpType.max, mybir.AluOpType.add,
                                accum_out=S[:, s_col:s_col + 1],
                            )
                coff += cw
            nc.sync.dma_start(s_d[:], S[:])

    nc.compile()
    return nc


def _build_phase2():
    nc = bacc.Bacc("TRN2", target_bir_lowering=False, debug=False, num_devices=1)
    gm_d = nc.dram_tensor("gmean", [128, NSEG], F16, kind="ExternalInput").ap()
    w2_d = nc.dram_tensor("w2", [128, 128], F16, kind="ExternalInput").ap()
    w3_d = nc.dram_tensor("w3", [128, 128], F16, kind="ExternalInput").ap()
    w4_d = nc.dram_tensor("w4", [128, ODIM], F16, kind="ExternalInput").ap()
    b2_d = nc.dram_tensor("b2", [128, 1], F32, kind="ExternalInput").ap()
    b3_d = nc.dram_tensor("b3", [128, 1], F32, kind="ExternalInput").ap()
    b4_d = nc.dram_tensor("b4", [ODIM, 1], F32, kind="ExternalInput").ap()
    out_d = nc.dram_tensor("out_t", [ODIM, NSEG], F32, kind="ExternalOutput").ap()

    with tile.TileContext(nc) as tc:
        with tc.tile_pool(name="sb", bufs=1) as pool, \
             tc.tile_pool(name="ps", bufs=6, space="PSUM") as psp:
            gm = pool.tile([128, NSEG], F16)
            nc.sync.dma_start(gm[:], gm_d[:])
            w2 = pool.tile([128, 128], F16)
            nc.sync.dma_start(w2[:], w2_d[:])
            w3 = pool.tile([128, 128], F16)
            nc.sync.dma_start(w3[:], w3_d[:])
            w4 = pool.tile([128, ODIM], F16)
            nc.sync.dma_start(w4[:], w4_d[:])
            b2 = pool.tile([128, 1], F32)
            nc.sync.dma_start(b2[:], b2_d[:])
            b3 = pool.tile([128, 1], F32)
            nc.sync.dma_start(b3[:], b3_d[:])
            b4 = pool.tile([ODIM, 1], F32)
            nc.sync.dma_start(b4[:], b4_d[:])

            hid = pool.tile([128, NSEG], F16)
            t3 = pool.tile([128, NSEG], F16)
            ot = pool.tile([ODIM, NSEG], F32)
            for j in range(NSEG // 512):
                sl = slice(512 * j, 512 * j + 512)
                p2 = psp.tile([128, 512], F32, tag="p")
                nc.tensor.matmul(p2[:], lhsT=w2[:], rhs=gm[:, sl],
                                 start=True, stop=True)
                nc.scalar.activation(out=hid[:, sl], in_=p2[:],
                                     func=mybir.ActivationFunctionType.Identity,
                                     bias=b2[:, 0:1])
            for j in range(NSEG // 512):
                sl = slice(512 * j, 512 * j + 512)
                p3 = psp.tile([128, 512], F32, tag="p")
                nc.tensor.matmul(p3[:], lhsT=w3[:], rhs=hid[:, sl],
                                 start=True, stop=True)
                nc.scalar.activation(out=t3[:, sl], in_=p3[:],
                                     func=mybir.ActivationFunctionType.Relu,
                                     bias=b3[:, 0:1])
            for j in range(NSEG // 512):
                sl = slice(512 * j, 512 * j + 512)
                p4f = psp.tile([128, 512], F32, tag="p")
                p4 = p4f[:ODIM, :]
                nc.tensor.matmul(p4, lhsT=w4[:], rhs=t3[:, sl],
                                 start=True, stop=True)
                nc.scalar.activation(out=ot[:, sl], in_=p4,
                                     func=mybir.ActivationFunctionType.Identity,
                                     bias=b4[:, 0:1])
            nc.sync.dma_start(out_d[:], ot[:])
    nc.compile()
    return nc


def run(inputs, ncores=NCORES, trace=False):
    x = np.asarray(inputs["x"], dtype=np.float32)
    xb = np.asarray(inputs["x_batch"])
    W1 = np.asarray(inputs["W1"], dtype=np.float32)
    b1 = np.asarray(inputs["b1"], dtype=np.float32)
    in_maps, meta = _host_prep(x, xb, W1, b1, ncores=ncores)

    nc1 = _build_phase1(meta)
    res1 = run_bass_kernel_spmd(nc1, in_maps, core_ids=list(range(ncores)),
                                trace=trace)

    # Host: route block sums to segments, cross-core reduce, bias/pad fixups
    gsumsT = np.zeros((NSEG, 128), dtype=np.float64)
    padA = np.zeros(NSEG, dtype=np.float64)
    padD = np.zeros(NSEG, dtype=np.float64)
    realD = np.zeros(NSEG, dtype=np.float64)
    for c in range(ncores):
        Sc = res1.results[c]["S"].astype(np.float64)
        bmap = meta["block_maps"][c]
        s_cols = np.array([b[0] for b in bmap], dtype=np.int64)
        segs = np.array([b[1] for b in bmap], dtype=np.int64)
        np.add.at(gsumsT, segs, Sc[:, s_cols].T)
        for s_col, seg, real, pad, eng in bmap:
            if eng == 0:
                padA[seg] += pad
            else:
                padD[seg] += pad
                realD[seg] += real
    b1d = b1.astype(np.float64)
    gsumsT += np.outer(realD, b1d)                       # DVE real cols: +b1
    gsumsT -= np.outer(padA, np.maximum(b1d, 0.0))       # ACT pads: relu(b1)
    gsumsT -= np.outer(padD, np.maximum(-b1d, 0.0))      # DVE pads: relu(-b1)

    counts = meta["counts"]
    invc = 1.0 / np.maximum(counts, 1.0)
    gmean = (gsumsT * invc[:, None]).T.astype(np.float16)

    p2_ins = [dict(
        gmean=np.ascontiguousarray(gmean),
        w2=np.ascontiguousarray(inputs["W2"], dtype=np.float16),
        w3=np.ascontiguousarray(inputs["W3"], dtype=np.float16),
        w4=np.ascontiguousarray(inputs["W4"], dtype=np.float16),
        b2=np.ascontiguousarray(inputs["b2"], dtype=np.float32).reshape(128, 1),
        b3=np.ascontiguousarray(inputs["b3"], dtype=np.float32).reshape(128, 1),
        b4=np.ascontiguousarray(inputs["b4"], dtype=np.float32).reshape(ODIM, 1),
    )]
    nc2 = _build_phase2()
    res2 = run_bass_kernel_spmd(nc2, p2_ins, core_ids=[0], trace=trace)
    out = np.ascontiguousarray(res2.results[0]["out_t"].T).astype(np.float32)
    # Empty segments: reference gives hid = 0 (b2 enters per-node, so an
    # empty sum has no b2), but the device kernel adds b2 unconditionally.
    empty = counts == 0
    if empty.any():
        b3 = np.asarray(inputs["b3"], dtype=np.float64)
        W4 = np.asarray(inputs["W4"], dtype=np.float64)
        b4 = np.asarray(inputs["b4"], dtype=np.float64)
        out[empty] = (np.maximum(b3, 0.0) @ W4 + b4).astype(np.float32)
    return out, res1, res2


def kernel(**inputs):
    inputs = {k: np.asarray(v) for k, v in inputs.items()}
    out, _, _ = run(inputs)
    return out


if __name__ == "__main__":
    rng = np.random.default_rng(0)
    N, D, HN, B = 8 * 2 * 4 * GW, 64, 128, 64
    x = rng.standard_normal((N, D), dtype=np.float32)
    xb = np.sort(rng.integers(0, B, N).astype(np.int32))
    W1 = (rng.standard_normal((D, HN)) / 8).astype(np.float32)
    W2 = (rng.standard_normal((HN, HN)) / 11.3).astype(np.float32)
    W3 = (rng.standard_normal((HN, HN)) / 11.3).astype(np.float32)
    W4 = (rng.standard_normal((HN, ODIM)) / 11.3).astype(np.float32)
    b1 = rng.standard_normal(HN).astype(np.float32) * 0.1
    b2 = rng.standard_normal(HN).astype(np.float32) * 0.1
    b3 = rng.standard_normal(HN).astype(np.float32) * 0.1
    b4 = rng.standard_normal(ODIM).astype(np.float32) * 0.1
    ins = dict(x=x, x_batch=xb, W1=W1, b1=b1, W2=W2, b2=b2, W3=W3, b3=b3,
               W4=W4, b4=b4)
    out = kernel(**ins)

    h = np.maximum(x @ W1 + b1, 0) @ W2 + b2
    sums = np.zeros((1024, HN), dtype=np.float64)
    np.add.at(sums, xb, h.astype(np.float64))
    cnt = np.bincount(xb, minlength=1024).astype(np.float64)
    mean = sums / np.maximum(cnt, 1)[:, None]
    ref = (np.maximum(mean @ W3 + b3, 0) @ W4 + b4).astype(np.float32)
    num = np.linalg.norm(out - ref)
    den = np.linalg.norm(ref)
    print("Relative error:", num / den)


# revision 10
# speedup vs baseline: 1.9032x; 1.0021x over previous
"""DeepSet segment-reduce kernel for 8 Trainium2 NeuronCores.

Math (equivalent to the reference, using linearity of segment_sum):
    r      = relu(x @ W1 + b1)                      # per-node, on device
    sums_r = segment_sum(r)                         # [B, HID]
    mean_r = sums_r / max(counts, 1)                # counts via host bincount
    hid    = mean_r @ W2 + b2                       # tiny tail, on device
    out    = relu(hid @ W3 + b3) @ W4 + b4          # tiny tail, on device

Device layout: hid on partitions, nodes on the free dim.  Each core gets a
contiguous shard of nodes split into two halves packed on partition halves
(features of half A in partitions 0..63, half B in 64..127), so DMA runs at
full 128-partition width.  Inputs are cast to fp16 on the host: the PE
streams fp16 at 1 col/cycle (4x the fp32 rate) and DMA traffic halves.

Segment sums: the host reorders each half so every segment's node run is
zero-padded to a multiple of 1024 columns.  Every 1024-column block then
belongs to exactly one segment.  Per block, two 512-col matmuls (PSUM bank
limit) produce h = W1^T x in PSUM, then ONE fused instruction computes the
block's relu sum into a column of S:
  - ScalarE blocks: activation(Relu, bias=b1, accum_out=S[:,col])
  - VectorE blocks: tensor_scalar(max(h, -b1), reduce add, accum_out)
    using relu(h + b1) = max(h, -b1) + b1; the  +b1*real_count  and the
    zero-pad columns' contributions are corrected exactly on the host.
The two engines work alternating blocks in disjoint PSUM bank regions
(ScalarE banks 0-3, VectorE banks 4-7), in parallel with the PE and DMA.
S [128, 2*nbh] is DMA'd out; segment routing, the cross-core reduction,
bias corrections and the mean all happen on the host (exact, cheap).
A second tiny single-core NEFF applies the rho MLP.
"""

import os
import sys

for _p in ("/opt/trn_rl_repo",):
    if os.path.isdir(_p) and _p not in sys.path:
        sys.path.append(_p)

import numpy as np

import concourse.bass as bass
import concourse.tile as tile
from concourse import bacc, mybir
from concourse.bass_utils import run_bass_kernel_spmd

F32 = mybir.dt.float32
F16 = mybir.dt.float16

NCORES = 8
GW = 1024           # columns per elementwise block (2 PSUM banks)
MW = 512            # columns per matmul (1 PSUM bank)
CW = 8192           # columns per DMA chunk
NSEG = 1024
ODIM = 16
HID = 128
NDIM = 64


def _engine_of(h, gb):
    """Block (half h, global block index gb) -> 0 (ScalarE) or 1 (VectorE).

    Must be identical between host prep and kernel build, and depend only on
    position (the SPMD NEFF is shared by all cores)."""
    return (gb + h) % 2


def _host_prep(x, x_batch, W1, b1, ncores=NCORES):
    N, _ = x.shape
    xb = np.asarray(x_batch)

    counts = np.bincount(xb, minlength=NSEG).astype(np.float64)

    # Run-length decomposition of the (sorted) x_batch, then balanced
    # assignment of whole segments to the 16 (core, half) bins so every bin
    # has nearly the same padded block count.
    uniq, starts = np.unique(xb, return_index=True)
    ends = np.append(starts[1:], len(xb))
    lens = ends - starts
    nblk = -(-lens // GW)
    order = np.argsort(-nblk, kind="stable")
    nbins = 2 * ncores
    bin_blocks = np.zeros(nbins, dtype=np.int64)
    bin_segs = [[] for _ in range(nbins)]
    for k in order:
        b = int(np.argmin(bin_blocks))
        bin_segs[b].append(int(k))
        bin_blocks[b] += nblk[k]

    nbh = int(bin_blocks.max())
    cols = nbh * GW
    sw = 2 * nbh

    w1d = np.vstack([W1, W1]).astype(np.float16)
    b1f = np.ascontiguousarray(b1, np.float32).reshape(HID, 1)
    nb1f = np.ascontiguousarray(-b1, np.float32).reshape(HID, 1)

    in_maps = []
    block_maps = []     # per core: list of (s_col, seg, real, pad, eng)
    for c in range(ncores):
        xt = np.zeros((128, cols), dtype=np.float16)
        bmap = []
        for h in range(2):
            src = np.full(cols, -1, dtype=np.int64)
            col = 0
            for k in bin_segs[2 * c + h]:
                L = int(lens[k])
                nb = -(-L // GW)
                src[col:col + L] = np.arange(starts[k], ends[k])
                for j in range(nb):
                    gb = col // GW + j
                    real = min(GW, L - j * GW)
                    bmap.append((h * nbh + gb, int(uniq[k]), real,
                                 GW - real, _engine_of(h, gb)))
                col += nb * GW
            mask = src >= 0
            gath = np.zeros((cols, NDIM), dtype=np.float16)
            gath[mask] = x[src[mask]].astype(np.float16)
            xt[64 * h:64 * h + 64, :] = gath.T
        in_maps.append(dict(xt=xt, w1d=w1d, b1=b1f, nb1=nb1f))
        block_maps.append(bmap)

    meta = dict(cols=cols, nbh=nbh, sw=sw, ncores=ncores,
                counts=counts, block_maps=block_maps)
    return in_maps, meta


def _build_phase1(meta):
    cols, nbh, sw = meta["cols"], meta["nbh"], meta["sw"]
    ncores = meta["ncores"]
    cw_list = [CW] * (cols // CW)
    if cols % CW:
        cw_list.append(cols % CW)

    nc = bacc.Bacc("TRN2", target_bir_lowering=False, debug=False,
                   num_devices=ncores)
    xt_d = nc.dram_tensor("xt", [128, cols], F16, kind="ExternalInput").ap()
    w1_d = nc.dram_tensor("w1d", [128, 128], F16, kind="ExternalInput").ap()
    b1_d = nc.dram_tensor("b1", [128, 1], F32, kind="ExternalInput").ap()
    nb1_d = nc.dram_tensor("nb1", [128, 1], F32, kind="ExternalInput").ap()
    s_d = nc.dram_tensor("S", [128, sw], F32, kind="ExternalOutput").ap()

    with tile.TileContext(nc) as tc:
        with tc.tile_pool(name="const", bufs=1) as cpool, \
             tc.tile_pool(name="xin", bufs=3) as xpool, \
             tc.tile_pool(name="sp", bufs=1) as spool, \
             tc.tile_pool(name="ps", bufs=1, space="PSUM") as pspool:

            w1t = cpool.tile([128, 128], F16)
            nc.sync.dma_start(w1t[:], w1_d[:])
            b1t = cpool.tile([128, 1], F32)
            nc.sync.dma_start(b1t[:], b1_d[:])
            nb1t = cpool.tile([128, 1], F32)
            nc.sync.dma_start(nb1t[:], nb1_d[:])
            S = spool.tile([128, sw], F32)
            ps = pspool.tile([128, 4096], F32)
            trash = spool.tile([128, 4, GW], mybir.dt.bfloat16)

            # The PE array holds both halves' 64x128 weight tiles resident
            # for the whole kernel (disjoint row strips); the tile
            # scheduler's per-matmul LDWEIGHTS (which serializes with the
            # ifmap stream on the xbus, ~184ns per matmul) are stripped
            # from the block after scheduling, keeping only these two.
            ldw_keep = {
                nc.tensor.ldweights(w1t[0:64, :], tile_position=(0, 0)).ins.name,
                nc.tensor.ldweights(w1t[64:128, :], tile_position=(64, 0)).ins.name,
            }

            slot_ctr = [0, 0]   # per-engine ping-pong
            coff = 0
            for cw in cw_list:
                bpc = cw // GW
                xch = xpool.tile([128, CW], F16, tag="xch")
                nc.sync.dma_start(xch[:, :cw], xt_d[:, coff:coff + cw])
                for h in range(2):
                    for blk in range(bpc):
                        gb = coff // GW + blk
                        s_col = h * nbh + gb
                        eng = _engine_of(h, gb)
                        slot = slot_ctr[eng] % 2
                        slot_ctr[eng] += 1
                        base = eng * 2048 + slot * GW
                        for s2 in range(GW // MW):
                            mm = nc.tensor.matmul(
                                ps[:, base + s2 * MW: base + (s2 + 1) * MW],
                                lhsT=w1t[64 * h:64 * h + 64, :],
                                rhs=xch[64 * h:64 * h + 64,
                                        blk * GW + s2 * MW: blk * GW + (s2 + 1) * MW],
                                start=True, stop=True,
                                tile_position=(64 * h, 0),
                            )
                            mm.ins.ldweights = False
                        region = ps[:, base:base + GW]
                        if eng == 0:
                            nc.scalar.activation(
                                out=region, in_=region,
                                func=mybir.ActivationFunctionType.Relu,
                                bias=b1t[:, 0:1],
                                accum_out=S[:, s_col:s_col + 1],
                            )
                        else:
                            tsl = slot_ctr[1] % 4
                            nc.vector.tensor_scalar(
                                trash[:, tsl, :], region, nb1t[:, 0:1], None,
                                mybir.AluOpType.max, mybir.AluOpType.add,
                                accum_out=S[:, s_col:s_col + 1],
                            )
                coff += cw
            nc.sync.dma_start(s_d[:], S[:])

    # Strip the scheduler-inserted per-matmul LDWEIGHTS: the weights are
    # resident for the whole kernel (loaded by the two kept LDWs), and the
    # inserted ones carry no semaphores (all sync lives on the matmuls).
    for f in nc.m.functions:
        for bb in f.blocks:
            il = bb.instructions
            keep = [i for i in il
                    if not (type(i).__name__ == "InstLdweights"
                            and i.sync_info is None
                            and i.name not in ldw_keep)]
            if len(keep) != len(il):
                bb.instructions = keep

    nc.compile()
    return nc


def _build_phase2():
    nc = bacc.Bacc("TRN2", target_bir_lowering=False, debug=False, num_devices=1)
    gm_d = nc.dram_tensor("gmean", [128, NSEG], F16, kind="ExternalInput").ap()
    w2_d = nc.dram_tensor("w2", [128, 128], F16, kind="ExternalInput").ap()
    w3_d = nc.dram_tensor("w3", [128, 128], F16, kind="ExternalInput").ap()
    w4_d = nc.dram_tensor("w4", [128, ODIM], F16, kind="ExternalInput").ap()
    b2_d = nc.dram_tensor("b2", [128, 1], F32, kind="ExternalInput").ap()
    b3_d = nc.dram_tensor("b3", [128, 1], F32, kind="ExternalInput").ap()
    b4_d = nc.dram_tensor("b4", [ODIM, 1], F32, kind="ExternalInput").ap()
    out_d = nc.dram_tensor("out_t", [ODIM, NSEG], F32, kind="ExternalOutput").ap()

    with tile.TileContext(nc) as tc:
        with tc.tile_pool(name="sb", bufs=1) as pool, \
             tc.tile_pool(name="ps", bufs=6, space="PSUM") as psp:
            gm = pool.tile([128, NSEG], F16)
            nc.sync.dma_start(gm[:], gm_d[:])
            w2 = pool.tile([128, 128], F16)
            nc.sync.dma_start(w2[:], w2_d[:])
            w3 = pool.tile([128, 128], F16)
            nc.sync.dma_start(w3[:], w3_d[:])
            w4 = pool.tile([128, ODIM], F16)
            nc.sync.dma_start(w4[:], w4_d[:])
            b2 = pool.tile([128, 1], F32)
            nc.sync.dma_start(b2[:], b2_d[:])
            b3 = pool.tile([128, 1], F32)
            nc.sync.dma_start(b3[:], b3_d[:])
            b4 = pool.tile([ODIM, 1], F32)
            nc.sync.dma_start(b4[:], b4_d[:])

            hid = pool.tile([128, NSEG], F16)
            t3 = pool.tile([128, NSEG], F16)
            ot = pool.tile([ODIM, NSEG], F32)
            for j in range(NSEG // 512):
                sl = slice(512 * j, 512 * j + 512)
                p2 = psp.tile([128, 512], F32, tag="p")
                nc.tensor.matmul(p2[:], lhsT=w2[:], rhs=gm[:, sl],
                                 start=True, stop=True)
                nc.scalar.activation(out=hid[:, sl], in_=p2[:],
                                     func=mybir.ActivationFunctionType.Identity,
                                     bias=b2[:, 0:1])
            for j in range(NSEG // 512):
                sl = slice(512 * j, 512 * j + 512)
                p3 = psp.tile([128, 512], F32, tag="p")
                nc.tensor.matmul(p3[:], lhsT=w3[:], rhs=hid[:, sl],
                                 start=True, stop=True)
                nc.scalar.activation(out=t3[:, sl], in_=p3[:],
                                     func=mybir.ActivationFunctionType.Relu,
                                     bias=b3[:, 0:1])
            for j in range(NSEG // 512):
                sl = slice(512 * j, 512 * j + 512)
                p4f = psp.tile([128, 512], F32, tag="p")
                p4 = p4f[:ODIM, :]
                nc.tensor.matmul(p4, lhsT=w4[:], rhs=t3[:, sl],
                                 start=True, stop=True)
                nc.scalar.activation(out=ot[:, sl], in_=p4,
                                     func=mybir.ActivationFunctionType.Identity,
                                     bias=b4[:, 0:1])
            nc.sync.dma_start(out_d[:], ot[:])
    nc.compile()
    return nc


def run(inputs, ncores=NCORES, trace=False):
    x = np.asarray(inputs["x"], dtype=np.float32)
    xb = np.asarray(inputs["x_batch"])
    W1 = np.asarray(inputs["W1"], dtype=np.float32)
    b1 = np.asarray(inputs["b1"], dtype=np.float32)
    in_maps, meta = _host_prep(x, xb, W1, b1, ncores=ncores)

    nc1 = _build_phase1(meta)
    res1 = run_bass_kernel_spmd(nc1, in_maps, core_ids=list(range(ncores)),
                                trace=trace)

    # Host: route block sums to segments, cross-core reduce, bias/pad fixups
    gsumsT = np.zeros((NSEG, 128), dtype=np.float64)
    padA = np.zeros(NSEG, dtype=np.float64)
    padD = np.zeros(NSEG, dtype=np.float64)
    realD = np.zeros(NSEG, dtype=np.float64)
    for c in range(ncores):
        Sc = res1.results[c]["S"].astype(np.float64)
        bmap = meta["block_maps"][c]
        s_cols = np.array([b[0] for b in bmap], dtype=np.int64)
        segs = np.array([b[1] for b in bmap], dtype=np.int64)
        np.add.at(gsumsT, segs, Sc[:, s_cols].T)
        for s_col, seg, real, pad, eng in bmap:
            if eng == 0:
                padA[seg] += pad
            else:
                padD[seg] += pad
                realD[seg] += real
    b1d = b1.astype(np.float64)
    gsumsT += np.outer(realD, b1d)                       # DVE real cols: +b1
    gsumsT -= np.outer(padA, np.maximum(b1d, 0.0))       # ACT pads: relu(b1)
    gsumsT -= np.outer(padD, np.maximum(-b1d, 0.0))      # DVE pads: relu(-b1)

    counts = meta["counts"]
    invc = 1.0 / np.maximum(counts, 1.0)
    gmean = (gsumsT * invc[:, None]).T.astype(np.float16)

    p2_ins = [dict(
        gmean=np.ascontiguousarray(gmean),
        w2=np.ascontiguousarray(inputs["W2"], dtype=np.float16),
        w3=np.ascontiguousarray(inputs["W3"], dtype=np.float16),
        w4=np.ascontiguousarray(inputs["W4"], dtype=np.float16),
        b2=np.ascontiguousarray(inputs["b2"], dtype=np.float32).reshape(128, 1),
        b3=np.ascontiguousarray(inputs["b3"], dtype=np.float32).reshape(128, 1),
        b4=np.ascontiguousarray(inputs["b4"], dtype=np.float32).reshape(ODIM, 1),
    )]
    nc2 = _build_phase2()
    res2 = run_bass_kernel_spmd(nc2, p2_ins, core_ids=[0], trace=trace)
    out = np.ascontiguousarray(res2.results[0]["out_t"].T).astype(np.float32)
    # Empty segments: reference gives hid = 0 (b2 enters per-node, so an
    # empty sum has no b2), but the device kernel adds b2 unconditionally.
    empty = counts == 0
    if empty.any():
        b3 = np.asarray(inputs["b3"], dtype=np.float64)
        W4 = np.asarray(inputs["W4"], dtype=np.float64)
        b4 = np.asarray(inputs["b4"], dtype=np.float64)
        out[empty] = (np.maximum(b3, 0.0) @ W4 + b4).astype(np.float32)
    return out, res1, res2


def kernel(**inputs):
    inputs = {k: np.asarray(v) for k, v in inputs.items()}
    out, _, _ = run(inputs)
    return out


if __name__ == "__main__":
    rng = np.random.default_rng(0)
    N, D, HN, B = 8 * 2 * 4 * GW, 64, 128, 64
    x = rng.standard_normal((N, D), dtype=np.float32)
    xb = np.sort(rng.integers(0, B, N).astype(np.int32))
    W1 = (rng.standard_normal((D, HN)) / 8).astype(np.float32)
    W2 = (rng.standard_normal((HN, HN)) / 11.3).astype(np.float32)
    W3 = (rng.standard_normal((HN, HN)) / 11.3).astype(np.float32)
    W4 = (rng.standard_normal((HN, ODIM)) / 11.3).astype(np.float32)
    b1 = rng.standard_normal(HN).astype(np.float32) * 0.1
    b2 = rng.standard_normal(HN).astype(np.float32) * 0.1
    b3 = rng.standard_normal(HN).astype(np.float32) * 0.1
    b4 = rng.standard_normal(ODIM).astype(np.float32) * 0.1
    ins = dict(x=x, x_batch=xb, W1=W1, b1=b1, W2=W2, b2=b2, W3=W3, b3=b3,
               W4=W4, b4=b4)
    out = kernel(**ins)

    h = np.maximum(x @ W1 + b1, 0) @ W2 + b2
    sums = np.zeros((1024, HN), dtype=np.float64)
    np.add.at(sums, xb, h.astype(np.float64))
    cnt = np.bincount(xb, minlength=1024).astype(np.float64)
    mean = sums / np.maximum(cnt, 1)[:, None]
    ref = (np.maximum(mean @ W3 + b3, 0) @ W4 + b4).astype(np.float32)
    num = np.linalg.norm(out - ref)
    den = np.linalg.norm(ref)
    print("Relative error:", num / den)
